# revision 2
# baseline (speedup 1.0000x reference)
"""Trainium2 Bass kernel for AttentionAggregator (GNN message passing).

Reference computation:
    new_emb = fb @ W + b
    s_e     = (fa @ a1)[src_e] + (new_emb @ a2)[dst_e]
    score_e = exp(elu(s_e, 0.1))
    out[n]  = (sum_{e: src_e=n} score_e * new_emb[dst_e]) / max(den[n], den==0->1)

Algebraic reformulation used here (linearity of the segment sum):
    q_e   = fb[dst_e] @ (W @ a2)            # per-edge scalar
    s_e   = (fa @ a1)[src_e] + q_e + b @ a2
    G[n]  = sum_e score_e * fb[dst_e]       # [Na, 64]
    den[n]= sum_e score_e
    out[n]= (G[n] / den_safe[n]) @ W + 1[den[n] > 0] * b

so new_emb is never materialized; only raw fb rows are gathered.

Distribution: nodes (and their incoming edge lists, after a host-side sort of
edges by src) are sharded contiguously across the 8 cores.  Each core owns
6272 output rows, gathers fb rows for its own edges (fb replicated), and no
cross-core collective is needed.

Device-side layout: each node's edge list is split into "virtual nodes" of at
most D0=24 slots.  A group of 128 virtual nodes occupies the 128 partitions;
their slot indices are gathered with one multi-index indirect DMA
([128, B*24] indices -> [128, B*24*64] fb rows for B groups per call).  Slot
scores are computed with per-slot ops, and the slot dimension is reduced with
strided-AP tensor_reduce.  A second tiny pass combines the <=KV virtual rows
of each real node, divides by den, and applies the final @W (+b) with the PE.
"""

import sys

for _p in ("/opt/trn_rl_repo",):
    if _p not in sys.path:
        sys.path.insert(0, _p)

import numpy as np

import concourse.bass as bass
import concourse.bacc as bacc
import concourse.mybir as mybir
import concourse.tile as tile
from concourse.bass import IndirectOffsetOnAxis
from concourse.masks import make_identity

P = 128
F = 64          # feature dim
D0 = 12         # edge slots per virtual node
NCORES = 8

f32 = mybir.dt.float32
bf16 = mybir.dt.bfloat16
i32 = mybir.dt.int32
i16 = mybir.dt.int16
TC = 16          # nodes per partition in the table-build pass
TW = 128         # augmented table row (bf16): fb (64) | q (1) | pad -> 256B
AX = mybir.AxisListType
OP = mybir.AluOpType
ACTF = mybir.ActivationFunctionType


# ----------------------------------------------------------------------------
# device program
# ----------------------------------------------------------------------------

def emit_program(tc, ins, outs, cfg):
    """Emit the per-core program.

    ins:  dict of APs: fb_tab [NB, F], pk [Gv, P, PKW], cpk [Gc, P, 2*KV],
          wvec [P, 3*F], wmat [F, F]
    outs: dict of APs: out [Gc*P, F], vtab [Gvp*P, VW] (scratch, Internal)
    cfg:  dict(Gv, Gc, KV, B, ba2)
    """
    nc = tc.nc
    Gv, Gc, KV, B = cfg["Gv"], cfg["Gc"], cfg["KV"], cfg["B"]
    ba2 = float(cfg["ba2"])
    fb_tab = ins["fb_tab"]
    pk = ins["pk"]
    cpk = ins["cpk"]
    wvec = ins["wvec"]
    wmat = ins["wmat"]
    out = outs["out"]
    vtab = outs["vtab"]
    tab2 = outs["tab2"]
    NB_pad = fb_tab.shape[0]

    gidx = ins["gidx"]
    cidx = ins["cidx"]
    PKW = F + D0              # fa_v row | slot mask
    NIa, H = cfg["NIa"], cfg["H"]
    VW = 128                  # vtab row (f32): G (64) | den (1) | pad -> 512B
    S = B * D0                # slots per phase-1 iteration
    NI = Gv // B
    assert Gv % B == 0

    with (
        tc.tile_pool(name="const", bufs=1) as cpool,
        tc.tile_pool(name="work", bufs=4) as pool,
        tc.tile_pool(name="psum", bufs=3, space="PSUM") as psum,
    ):
        wvec_t = cpool.tile([P, 3 * F], f32)
        nc.sync.dma_start(out=wvec_t[:], in_=wvec)
        wmat_t = cpool.tile([P, F], f32)
        nc.sync.dma_start(out=wmat_t[0:F, :], in_=wmat)
        nc.sync.dma_start(out=wmat_t[F:2 * F, :], in_=wmat)
        ident = cpool.tile([P, P], f32)
        make_identity(nc, ident[:])
        zbias = cpool.tile([P, 1], f32)
        nc.vector.memset(zbias[:], 0.0)
        mbias = cpool.tile([P, 1], f32)
        nc.vector.memset(mbias[:], -0.1)

        a1v = wvec_t[:, 0:F]
        w2v = wvec_t[:, F:2 * F]
        bv = wvec_t[:, 2 * F:3 * F]
        w2b = cpool.tile([P, F], bf16)
        nc.vector.tensor_copy(out=w2b[:], in_=w2v)

        # ---------------- phase 0: build augmented bf16 table [fb | q] -----
        abl = cfg.get("ablate", set())
        NTI = NB_pad // (P * TC) if "p0" not in abl else 0
        # tiles 0..NTA-1 cover table half A (rows [0, H)); phase-1 A-batches
        # only read half A, so half-B tiles can build concurrently with them.
        NTA = min(NTI, -(-H // (P * TC)))
        fb4 = fb_tab.rearrange("(j p c) f -> j p c f", p=P, c=TC)
        t24 = tab2.rearrange("(j p c) w -> j p c w", p=P, c=TC)

        def build_tile(j):
            fbb = pool.tile([P, TC * F], bf16, tag="fbb")
            fbb3 = fbb[:].rearrange("p (c f) -> p c f", f=F)
            nc.gpsimd.dma_start(out=fbb3, in_=fb4[j])  # SWDGE cast f32->bf16
            prodt = pool.tile([P, TC * F], bf16, tag="prodt")
            prodt3 = prodt[:].rearrange("p (c f) -> p c f", f=F)
            nc.vector.tensor_tensor(
                out=prodt3, in0=fbb3,
                in1=w2b[:, None, :].to_broadcast([P, TC, F]), op=OP.mult,
            )
            qt = pool.tile([P, TC], f32, tag="qt")
            nc.vector.tensor_reduce(
                out=qt[:], in_=prodt3, axis=AX.X, op=OP.add,
            )
            pck = pool.tile([P, TC * TW], bf16, tag="pck")
            pck3 = pck[:].rearrange("p (c w) -> p c w", w=TW)
            nc.vector.tensor_copy(out=pck3[:, :, F:F + 1], in_=qt[:, :, None])
            nc.vector.tensor_copy(out=pck3[:, :, 0:F], in_=fbb3)
            nc.sync.dma_start(out=t24[j][:, :, 0:F + 1], in_=pck3[:, :, 0:F + 1])

        for j in range(NTI):
            build_tile(j)
        tc.strict_bb_all_engine_barrier()

        # ---------------- phase 1: per-virtual-node-group segment sums -----
        def phase1_iter(it):
            g0 = it * B
            pk_t = pool.tile([P, B * PKW], f32, tag="pk")
            nc.sync.dma_start(
                out=pk_t[:].rearrange("p (b w) -> p b w", w=PKW),
                in_=pk[g0:g0 + B].rearrange("b p w -> p b w"),
            )
            pk3 = pk_t[:].rearrange("p (b w) -> p b w", w=PKW)
            gi_t = pool.tile([P, S * P // 16], i16, tag="gi", bufs=4)
            nc.sync.dma_start(out=gi_t[:], in_=gidx[it])

            rows = pool.tile([P, S * TW], bf16, tag="rows", bufs=4)
            rows3 = rows[:].rearrange("p (s w) -> p s w", w=TW)  # [P, S, 128]
            half = tab2[0:H, :] if it < NIa else tab2[H:2 * H, :]
            NIDX = cfg.get("nidx", 1024)      # per-call ring-capacity limit
            off = 0
            while off < S * P and "gather" not in abl:
                n = min(NIDX, S * P - off)
                nc.gpsimd.dma_gather(
                    out_ap=rows3[:, off // P:(off + n) // P, :],
                    in_ap=half,
                    idxs_ap=gi_t[:, off // 16:(off + n) // 16],
                    num_idxs=n,
                    num_idxs_reg=n,
                    elem_size=TW,
                )
                off += n
            # e1[p, b] = fa_v[p, b, :] @ a1
            fprod = pool.tile([P, B * F], f32, tag="fprod")
            nc.vector.tensor_tensor(
                out=fprod[:].rearrange("p (b f) -> p b f", f=F),
                in0=pk3[:, :, 0:F],
                in1=a1v[:, None, :].to_broadcast([P, B, F]),
                op=OP.mult,
            )
            e1 = pool.tile([P, B], f32, tag="e1")
            nc.vector.tensor_reduce(
                out=e1[:],
                in_=fprod[:].rearrange("p (b f) -> p b f", f=F),
                axis=AX.X, op=OP.add,
            )
            if ba2 != 0.0:
                nc.vector.tensor_scalar(
                    out=e1[:], in0=e1[:], scalar1=ba2, scalar2=None, op0=OP.add,
                )

            # s = q + e1; q is the gathered bf16 column 64 (+ ba2 in e1)
            s_t = pool.tile([P, S], f32, tag="s")
            nc.vector.tensor_tensor(
                out=s_t[:].rearrange("p (b k) -> p b k", k=D0),
                in0=rows3[:, :, F].rearrange("p (b k) -> p b k", k=D0),
                in1=e1[:, :, None].to_broadcast([P, B, D0]),
                op=OP.add,
            )

            # score = where(s + ba2 > 0, exp(s + ba2), exp(0.1*exp(s+ba2) - 0.1))
            t_t = pool.tile([P, S], f32, tag="t")
            nc.scalar.activation(t_t[:], s_t[:], ACTF.Exp, bias=zbias[:, 0:1],
                                 scale=1.0)
            u_t = pool.tile([P, S], f32, tag="u")
            nc.scalar.activation(u_t[:], t_t[:], ACTF.Exp, bias=mbias[:, 0:1],
                                 scale=0.1)
            m_t = pool.tile([P, S], mybir.dt.uint8, tag="m")
            nc.vector.tensor_scalar(
                out=m_t[:], in0=s_t[:], scalar1=0.0, scalar2=None, op0=OP.is_gt,
            )
            nc.vector.copy_predicated(out=u_t[:], mask=m_t[:], data=t_t[:])
            # zero padded slots and downcast to bf16 in one op
            u2 = pool.tile([P, S], bf16, tag="u2")
            nc.vector.tensor_tensor(
                out=u2[:].rearrange("p (b k) -> p b k", k=D0),
                in0=u_t[:].rearrange("p (b k) -> p b k", k=D0),
                in1=pk3[:, :, F:F + D0],
                op=OP.mult,
            )
            scaled = pool.tile([P, S * F], bf16, tag="scaled", bufs=2)
            scaled3 = scaled[:].rearrange("p (s f) -> p s f", f=F)
            vout = pool.tile([P, B * VW], f32, tag="vout")
            vout3 = vout[:].rearrange("p (b w) -> p b w", w=VW)
            if "big" not in abl:
                nc.vector.tensor_tensor(
                    out=scaled3,
                    in0=rows3[:, :, 0:F],
                    in1=u2[:, :, None].to_broadcast([P, S, F]),
                    op=OP.mult,
                )
                nc.vector.tensor_reduce(
                    out=vout3[:, :, 0:F],
                    in_=scaled[:].rearrange("p (b k f) -> p b f k", k=D0, f=F),
                    axis=AX.X, op=OP.add,
                )
            nc.vector.tensor_reduce(
                out=vout3[:, :, F:F + 1],
                in_=u2[:].rearrange("p (b k) -> p b k", k=D0),
                axis=AX.X, op=OP.add,
            )
            nc.sync.dma_start(
                out=vtab.rearrange("(g p) w -> g p w", p=P)[g0:g0 + B]
                    .rearrange("b p w -> p b w")[:, :, 0:F + 1],
                in_=vout3[:, :, 0:F + 1],
            )

        for it in range(NI):
            phase1_iter(it)

        # ---------------- phase 2: combine virtual rows, divide, @W + b ----
        B2 = cfg["B2"]
        Gc2 = cfg["Gc2"]
        out3 = out.rearrange("(g p) f -> g p f", p=P)
        for r2 in range(Gc2 // B2 if cfg.get("phases", "all") == "all" else 0):
            r0 = r2 * B2
            cpk_t = pool.tile([P, B2 * KV], f32, tag="cpk")
            nc.sync.dma_start(
                out=cpk_t[:].rearrange("p (b k) -> p b k", k=KV),
                in_=cpk[r0:r0 + B2].rearrange("b p k -> p b k"),
            )
            cm = cpk_t[:, 0:B2 * KV]
            ci_t = pool.tile([P, B2 * KV * P // 16], i16, tag="ci")
            nc.sync.dma_start(out=ci_t[:], in_=cidx[r2])

            gr = pool.tile([P, B2 * KV * VW], f32, tag="gr")
            gr3 = gr[:].rearrange("p (k w) -> p k w", w=VW)   # [P, B2*KV, VW]
            nc.gpsimd.dma_gather(
                out_ap=gr3,
                in_ap=vtab,
                idxs_ap=ci_t[:],
                num_idxs=B2 * KV * P,
                num_idxs_reg=B2 * KV * P,
                elem_size=VW,
            )

            scm = pool.tile([P, B2 * KV * (F + 1)], f32, tag="scm")
            nc.vector.tensor_tensor(
                out=scm[:].rearrange("p (k w) -> p k w", w=F + 1),
                in0=gr3[:, :, 0:F + 1],
                in1=cm[:, :, None].to_broadcast([P, B2 * KV, F + 1]),
                op=OP.mult,
            )
            hd = pool.tile([P, B2 * (F + 1)], f32, tag="hd")
            hd3 = hd[:].rearrange("p (b w) -> p b w", w=F + 1)
            nc.vector.tensor_reduce(
                out=hd3,
                in_=scm[:].rearrange("p (b k w) -> p b w k", k=KV, w=F + 1),
                axis=AX.X, op=OP.add,
            )
            den = hd3[:, :, F]                                 # [P, B2]
            m0 = pool.tile([P, B2], f32, tag="m0")
            nc.vector.tensor_scalar(
                out=m0[:], in0=den, scalar1=0.0, scalar2=None, op0=OP.is_equal,
            )
            dsafe = pool.tile([P, B2], f32, tag="dsafe")
            nc.vector.tensor_tensor(out=dsafe[:], in0=den, in1=m0[:], op=OP.add)
            rec = pool.tile([P, B2], f32, tag="rec")
            nc.vector.reciprocal(rec[:], dsafe[:])
            h_t = pool.tile([P, B2 * F], f32, tag="h")
            nc.vector.tensor_tensor(
                out=h_t[:].rearrange("p (b f) -> p b f", f=F),
                in0=hd3[:, :, 0:F],
                in1=rec[:, :, None].to_broadcast([P, B2, F]),
                op=OP.mult,
            )
            w1 = pool.tile([P, B2], f32, tag="w1")
            nc.vector.tensor_scalar(
                out=w1[:], in0=den, scalar1=0.0, scalar2=None, op0=OP.is_gt,
            )
            outs_t = pool.tile([P, B2 * F], f32, tag="outs")
            for b0 in range(0, B2, 2):
                bw = min(2, B2 - b0)
                htp = psum.tile([bw * F, P], f32, tag="htp")
                nc.tensor.transpose(
                    out=htp[:], in_=h_t[:, b0 * F:(b0 + bw) * F],
                    identity=ident[:])
                ht = pool.tile([bw * F, P], f32, tag="ht")
                nc.vector.tensor_copy(out=ht[:], in_=htp[:])
                for bb in range(bw):
                    b = b0 + bb
                    op_t = psum.tile([P, F], f32, tag="op")
                    nc.tensor.matmul(out=op_t[:],
                                     lhsT=ht[bb * F:(bb + 1) * F, :],
                                     rhs=wmat_t[bb * F:(bb + 1) * F, :],
                                     start=True, stop=True)
                    badd = pool.tile([P, F], f32, tag="badd")
                    nc.vector.tensor_scalar(
                        out=badd[:], in0=bv, scalar1=w1[:, b:b + 1],
                        scalar2=None, op0=OP.mult,
                    )
                    nc.vector.tensor_tensor(
                        out=outs_t[:, b * F:(b + 1) * F], in0=op_t[:],
                        in1=badd[:], op=OP.add,
                    )
            nc.sync.dma_start(
                out=out3[r0:r0 + B2].rearrange("g p f -> p g f"),
                in_=outs_t[:].rearrange("p (b f) -> p b f", f=F),
            )


# ----------------------------------------------------------------------------
# host-side preparation
# ----------------------------------------------------------------------------

def prep_inputs(feature_a, feature_b, W, b, a_vec, edges, node_num_a,
                ncores=NCORES, d0=D0):
    """Shard + pad inputs for the SPMD program.  Index plumbing only (sort,
    bincount, padding); the only host arithmetic is the tiny parameter
    derivation Wa2 = W @ a2 (64x64 matvec) and ba2 = b @ a2."""
    fa = np.asarray(feature_a, np.float32)
    fb = np.asarray(feature_b, np.float32)
    W = np.asarray(W, np.float32)
    b = np.asarray(b, np.float32)
    a_vec = np.asarray(a_vec, np.float32).reshape(-1)
    edges = np.asarray(edges)
    NA = int(node_num_a)
    NB, Fdim = fb.shape
    assert Fdim == F and fa.shape[1] == F

    src = edges[:, 0].astype(np.int64)
    dst = edges[:, 1].astype(np.int64)

    NB_pad = -(-NB // (P * TC)) * (P * TC)
    fb_pad = np.zeros((NB_pad, F), np.float32)
    fb_pad[:NB] = fb

    a1 = a_vec[:F]
    a2 = a_vec[F:]
    Wa2 = (W @ a2).astype(np.float32)
    ba2 = float(b @ a2)

    H = NB_pad // 2
    hflag = (dst >= H).astype(np.int64)
    order = np.lexsort((hflag, src))
    ssrc = src[order]
    sdst = dst[order].astype(np.int64)
    shf = hflag[order]
    deg = np.bincount(ssrc, minlength=NA).astype(np.int64)
    degA = np.bincount(ssrc[shf == 0], minlength=NA).astype(np.int64)
    degB = deg - degA
    row_ptr = np.zeros(NA + 1, np.int64)
    np.cumsum(deg, out=row_ptr[1:])

    nodes_per_core = -(-NA // (ncores * P)) * P          # 6272
    Gc = nodes_per_core // P                             # 49
    nvA = -(-degA // d0)
    nvB = -(-degB // d0)
    KV = max(2, int((nvA + nvB).max()))
    B = 4

    def build_half(lo, hi, degH, nvH, edge_off):
        """Virtual nodes for one dst-half of one core's node range.
        edge_off[n] = first sorted-edge position of this half's run."""
        n_nodes = max(hi - lo, 0)
        node_ids = np.arange(lo, hi)
        nvc = nvH[lo:hi] if n_nodes else np.zeros(0, np.int64)
        Nv = int(nvc.sum())
        vnode = np.repeat(node_ids, nvc)
        vstart0 = np.concatenate([[0], np.cumsum(nvc)])[:-1]
        vrank = np.arange(Nv) - np.repeat(vstart0, nvc)
        pos = edge_off[vnode][:, None] + vrank[:, None] * d0 + np.arange(d0)[None, :]
        valid = (vrank[:, None] * d0 + np.arange(d0)[None, :]) < degH[vnode][:, None]
        posc = np.clip(pos, 0, max(len(sdst) - 1, 0))
        sidx = np.where(valid, sdst[posc] if len(sdst) else 0, 0).astype(np.int64)
        return dict(Nv=Nv, vnode=vnode, nvc=nvc, vstart0=vstart0,
                    sidx=sidx, valid=valid)

    offA = row_ptr[:-1]            # A-run starts at the node's run start
    offB = row_ptr[:-1] + degA     # B-run follows
    cores = []
    for c in range(ncores):
        lo = c * nodes_per_core
        hi = min(lo + nodes_per_core, NA)
        ha = build_half(lo, hi, degA, nvA, offA)
        hb = build_half(lo, hi, degB, nvB, offB)
        hb["sidx"] = np.where(hb["valid"], hb["sidx"] - H, 0)
        cores.append((ha, hb))

    def cdiv(a, b):
        return -(-a // b)

    maxA = max(1, max(h[0]["Nv"] for h in cores))
    maxB = max(h[1]["Nv"] for h in cores)
    GvA = cdiv(cdiv(maxA, P), B) * B
    GvB = cdiv(cdiv(maxB, P), B) * B if maxB > 0 else 0
    NIa = GvA // B
    Gv = GvA + GvB
    Nvp = Gv * P
    B2 = min(4, max(1, 1024 // (KV * P)))
    Gc2 = cdiv(Gc, B2) * B2

    in_maps = []
    PKW = F + d0
    S = B * d0
    NI = Gv // B
    for c in range(ncores):
        ha, hb = cores[c]
        pk = np.zeros((Nvp, PKW), np.float32)
        sidx_all = np.zeros((Nvp, d0), np.int64)
        for (h, base) in ((ha, 0), (hb, GvA * P)):
            Nv = h["Nv"]
            if Nv:
                pk[base:base + Nv, 0:F] = fa[h["vnode"]]
                pk[base:base + Nv, F:F + d0] = h["valid"].astype(np.float32)
                sidx_all[base:base + Nv] = h["sidx"]
        pk = pk.reshape(Gv, P, PKW)

        # int16 gather indices: per batch, flat[(b*d0+k)*128 + p] =
        # sidx[group g0+b, partition p, slot k]; sbuf wrap [16, S*128//16],
        # replicated to 128 partitions; stored bitcast-f32 inside pk so one
        # DMA loads fa_v + mask + idx.  Device reads pk3[:, :, F+D0:] as the
        # per-iteration [P, S*P//16] i16 block, so per-group cols must hold
        # that group's quarter of the batch block: columns [b*d0*8*(..)].
        sidx_g = sidx_all.reshape(Gv, P, d0)
        gidx16 = np.zeros((NI, P, S * P // 16), np.int16)
        for i in range(NI):
            blk = sidx_g[i * B:(i + 1) * B]              # [B, P, d0]
            flat = blk.transpose(0, 2, 1).reshape(-1)    # [(b k) p]
            sb = flat.reshape(S * P // 16, 16).T.astype(np.int16)
            gidx16[i] = np.tile(sb, (8, 1))
        assert sidx_all.max() < 32768

        cpka = np.zeros((Gc2 * P, KV), np.float32)
        cidxa = np.zeros((Gc2 * P, KV), np.int64)
        n_nodes = min(nodes_per_core, NA - c * nodes_per_core)
        if n_nodes > 0:
            nv_tot = ha["nvc"] + hb["nvc"]
            ks = np.arange(KV)[None, :]
            cvalid = ks < nv_tot[:, None]
            # first the node's A-virtual rows, then its B-virtual rows
            inA = ks < ha["nvc"][:, None]
            idxA = ha["vstart0"][:, None] + ks
            idxB = GvA * P + hb["vstart0"][:, None] + (ks - ha["nvc"][:, None])
            cidxv = np.where(cvalid, np.where(inA, idxA, idxB), 0)
            cpka[:n_nodes, 0:KV] = cvalid.astype(np.float32)
            cidxa[:n_nodes] = cidxv
        cpk = cpka.reshape(Gc2, P, KV)
        assert cidxa.max() < 32768
        # int16 wrap for phase-2 dma_gather, one batch of B2 groups per call:
        # flat[(b*KV + k)*128 + p] = cidx[group r0+b, p, k]
        cg = cidxa.reshape(Gc2, P, KV)
        NW = B2 * KV * P // 16
        cidx16 = np.zeros((Gc2 // B2, P, NW), np.int16)
        for r in range(Gc2 // B2):
            flat = cg[r * B2:(r + 1) * B2].transpose(0, 2, 1).reshape(-1)
            sb = flat.reshape(NW, 16).T.astype(np.int16)
            cidx16[r] = np.tile(sb, (8, 1))

        wvec = np.zeros((P, 3 * F), np.float32)
        wvec[:, 0:F] = a1[None, :]
        wvec[:, F:2 * F] = Wa2[None, :]
        wvec[:, 2 * F:3 * F] = b[None, :]

        in_maps.append(dict(
            fb_tab=fb_pad,
            pk=np.ascontiguousarray(pk),
            gidx=np.ascontiguousarray(gidx16),
            cpk=np.ascontiguousarray(cpk),
            cidx=np.ascontiguousarray(cidx16),
            wvec=wvec,
            wmat=np.ascontiguousarray(W),
        ))

    cfg = dict(Gv=Gv, Gc=Gc, Gc2=Gc2, B2=B2, KV=KV, B=B, ba2=ba2, NB=NB,
               NB_pad=NB_pad, NIa=NIa, H=H, Nvp=Nvp,
               nodes_per_core=nodes_per_core, NA=NA)
    return in_maps, cfg


def build_bass(cfg, ncores=NCORES):
    nc = bacc.Bacc("TRN2", target_bir_lowering=False, debug=False,
                   enable_asserts=False, num_devices=ncores)
    ins = dict(
        fb_tab=nc.dram_tensor("fb_tab", [cfg["NB_pad"], F], f32,
                              kind="ExternalInput").ap(),
        pk=nc.dram_tensor("pk", [cfg["Gv"], P, F + D0], f32,
                          kind="ExternalInput").ap(),
        gidx=nc.dram_tensor("gidx", [cfg["Gv"] // cfg["B"], P,
                                     cfg["B"] * D0 * P // 16], i16,
                            kind="ExternalInput").ap(),
        cpk=nc.dram_tensor("cpk", [cfg["Gc2"], P, cfg["KV"]], f32,
                           kind="ExternalInput").ap(),
        cidx=nc.dram_tensor("cidx", [cfg["Gc2"] // cfg["B2"], P,
                                     cfg["B2"] * cfg["KV"] * P // 16], i16,
                            kind="ExternalInput").ap(),
        wvec=nc.dram_tensor("wvec", [P, 3 * F], f32, kind="ExternalInput").ap(),
        wmat=nc.dram_tensor("wmat", [F, F], f32, kind="ExternalInput").ap(),
    )
    outs = dict(
        out=nc.dram_tensor("out", [cfg["Gc2"] * P, F], f32,
                           kind="ExternalOutput").ap(),
        vtab=nc.dram_tensor("vtab", [cfg["Nvp"], 128], f32,
                            kind="ExternalOutput").ap(),
        tab2=nc.dram_tensor("tab2", [cfg["NB_pad"], TW], bf16,
                            kind="ExternalOutput").ap(),
    )
    with tile.TileContext(nc) as tc:
        emit_program(tc, ins, outs, cfg)
    nc.compile()
    return nc


# ----------------------------------------------------------------------------
# entry point
# ----------------------------------------------------------------------------

def assemble_output(results, cfg):
    outs = [r["out"][:cfg["nodes_per_core"]] for r in results]
    return np.concatenate(outs, axis=0)[:cfg["NA"]].astype(np.float32)


def kernel_with_results(trace=False, **inputs):
    from concourse import bass_utils

    in_maps, cfg = prep_inputs(**inputs)
    nc = build_bass(cfg)
    res = bass_utils.run_bass_kernel_spmd(
        nc, in_maps, core_ids=list(range(NCORES)), trace=trace,
    )
    return assemble_output(res.results, cfg), res


def kernel(**inputs):
    return kernel_with_results(trace=False, **inputs)[0]


if __name__ == "__main__":
    np.random.seed(0)
    NA = NB = 50000
    E = 800000
    ins = dict(
        feature_a=np.random.randn(NA, F).astype(np.float32),
        feature_b=np.random.randn(NB, F).astype(np.float32),
        W=(np.random.randn(F, F) / 8).astype(np.float32),
        b=np.zeros(F, np.float32),
        a_vec=(np.random.randn(2 * F, 1) * 0.05).astype(np.float32),
        edges=np.stack([np.random.randint(0, NA, E),
                        np.random.randint(0, NB, E)], 1).astype(np.int64),
        node_num_a=NA,
    )
    out = kernel(**ins)
    print(out.shape, out.dtype)

